# revision 2
# baseline (speedup 1.0000x reference)
"""Trainium2 Bass kernel v2 for nn_AttnSeq2Seq (2-layer LSTM encoder + attention decoder).

Sharding: pure data parallelism - batch 1024 = 8 x 128, weights replicated.

Encoder (L steps, 2 fused layers, software-pipelined with a 1-step skew):
  per step t the PE queue is [z0(t) | tr-h1(t-2) | z1(t-1) | tr-h0(t)] so the
  gate latency (ACT+DVE) of each layer hides under the other layer's matmuls.
  Gate columns are permuted to [i|f|o|g]: one Sigmoid over 1152 cols + one Tanh.
  enc_out (h1) is written to fp8: d-chunks 0,1 stay resident in SBUF as
  [d, b, l]; d-chunk 2 and the [l, b, d] layout are spilled to DRAM.
Decoder (HZ steps): scores via 4-way column-tiled M=1 matmuls (4 examples run
  concurrently in separate 32-column PE groups) against fp8 enc, softmax
  normalized at the source (ACT accum_out + per-partition reciprocal scale),
  alpha/ctx transposed via DRAM-round-trip dma_start_transpose, ctx via
  col-tiled M=1 matmuls against the fp8 [l, b, d] stream. No DVE inner loops,
  no per-example partition broadcasts.
"""
import os
import numpy as np
from contextlib import ExitStack

import concourse.bass as bass
import concourse.tile as tile
from concourse import bacc, mybir, bass_utils, masks
from concourse.tile import add_dep_helper

f32 = mybir.dt.float32
f16 = mybir.dt.float16
f8 = mybir.dt.float8e4
AF = mybir.ActivationFunctionType
OP = mybir.AluOpType

B, DX, H = 1024, 8, 384
L = int(os.environ.get("K_L", "336"))
HZ = int(os.environ.get("K_HZ", "18"))
NC = 8
BL = B // NC          # 128 per core
G4 = 4 * H            # 1536
SIG = 3 * H           # sigmoid span after [i|f|o|g] permutation
SPI = 16              # encoder steps per x/flush group
N_GROUPS = L // SPI
assert L % SPI == 0
TR_MODE = os.environ.get("TR_MODE", "pe")  # encoder h transposes: 'pe' | 'dma'
N_LC = (L + 127) // 128
L_CHUNKS = [(c * 128, min(128, L - 128 * c)) for c in range(N_LC)]

_cache = {}


def _build():
    nc = bacc.Bacc("TRN2", target_bir_lowering=False, debug=False)

    # ---------------- DRAM I/O ----------------
    d_xT = nc.dram_tensor("xT", [L, DX + 1, BL], f16, kind="ExternalInput").ap()
    d_wih0 = nc.dram_tensor("wih0", [DX + 1, G4], f16, kind="ExternalInput").ap()
    d_whh0 = nc.dram_tensor("whh0", [3, 128, G4], f16, kind="ExternalInput").ap()
    d_wih1 = nc.dram_tensor("wih1", [3, 128, G4], f16, kind="ExternalInput").ap()
    d_whh1 = nc.dram_tensor("whh1", [3, 128, G4], f16, kind="ExternalInput").ap()
    d_bias1 = nc.dram_tensor("bias1", [1, G4], f16, kind="ExternalInput").ap()
    d_wa = nc.dram_tensor("wa", [3, 3, 128, 128], f16, kind="ExternalInput").ap()
    d_dinw = nc.dram_tensor("dinw", [3, 128, H], f16, kind="ExternalInput").ap()
    d_dinwt = nc.dram_tensor("dinwt", [6, H], f16, kind="ExternalInput").ap()
    d_dwih = nc.dram_tensor("dwih", [3, 128, G4], f16, kind="ExternalInput").ap()
    d_dwhh = nc.dram_tensor("dwhh", [3, 128, G4], f16, kind="ExternalInput").ap()
    d_dbias = nc.dram_tensor("dbias", [1, G4], f16, kind="ExternalInput").ap()
    d_outw = nc.dram_tensor("outw", [3, 128, 1], f16, kind="ExternalInput").ap()
    d_outb = nc.dram_tensor("outb", [1, 1], f32, kind="ExternalInput").ap()
    d_featT = nc.dram_tensor("featT", [HZ, 4, BL], f16, kind="ExternalInput").ap()
    d_y = nc.dram_tensor("y", [BL, HZ], f32, kind="ExternalOutput").ap()
    # device-written scratch
    d_encT2 = nc.dram_tensor("encT2", [128, BL, L], f8, kind="ExternalOutput").ap()
    d_encL = nc.dram_tensor("encL", [L, BL, H], f8, kind="ExternalOutput").ap()

    with tile.TileContext(nc) as tc, ExitStack() as ctx:
        wp = ctx.enter_context(tc.tile_pool(name="wpers", bufs=1))
        st = ctx.enter_context(tc.tile_pool(name="state", bufs=1))
        gp = ctx.enter_context(tc.tile_pool(name="gates", bufs=2))
        ps = ctx.enter_context(tc.tile_pool(name="psum", bufs=2, space="PSUM"))
        enc_es = ExitStack()
        xp = enc_es.enter_context(tc.tile_pool(name="xin", bufs=2))
        fb = enc_es.enter_context(tc.tile_pool(name="flush", bufs=2))
        we = enc_es.enter_context(tc.tile_pool(name="wenc", bufs=1))

        # ---------------- persistent consts ----------------
        ident = wp.tile([128, 128], f32)
        masks.make_identity(nc, ident[:])
        ident16 = wp.tile([128, 128], f16)
        masks.make_identity(nc, ident16[:])
        ones1 = wp.tile([1, 128], f16, name="ones1")
        nc.gpsimd.memset(ones1[:], 1.0)
        # resident fp8 enc_out d-chunks 0,1: [128 d, BL, L]
        encres = [wp.tile([128, BL * L], f8, tag=f"encres{k}", name=f"encres{k}") for k in range(2)]
        encres_v = [t[:].rearrange("p (b l) -> p b l", b=BL) for t in encres]

        # ---------------- encoder-only weights (freed before decoder) --------
        wih0 = we.tile([DX + 1, G4], f16, name="wih0")
        nc.sync.dma_start(wih0[:], d_wih0)
        whh0 = [we.tile([128, G4], f16, tag=f"whh0{k}", name=f"whh0{k}") for k in range(3)]
        wih1 = [we.tile([128, G4], f16, tag=f"wih1{k}", name=f"wih1{k}") for k in range(3)]
        whh1 = [we.tile([128, G4], f16, tag=f"whh1{k}", name=f"whh1{k}") for k in range(3)]
        for k in range(3):
            nc.sync.dma_start(whh0[k][:], d_whh0[k])
            nc.sync.dma_start(wih1[k][:], d_wih1[k])
            nc.sync.dma_start(whh1[k][:], d_whh1[k])
        bias1 = we.tile([1, G4], f16, name="bias1")
        nc.sync.dma_start(bias1[:], d_bias1)

        # ---------------- state ----------------
        h0T = [[st.tile([128, 128], f16, tag=f"h0T{p}{k}", name=f"h0T{p}{k}") for k in range(3)]
               for p in range(2)]
        h1T = [[st.tile([128, 128], f16, tag=f"h1T{p}{k}", name=f"h1T{p}{k}") for k in range(3)]
               for p in range(2)]
        hdT = [[st.tile([128, 128], f16, tag=f"hdT{p}{k}", name=f"hdT{p}{k}") for k in range(3)]
               for p in range(2)]
        c0 = st.tile([128, H], f32, name="c0")
        c1 = st.tile([128, H], f32, name="c1")
        for p in range(2):
            for k in range(3):
                nc.gpsimd.memset(h0T[p][k][:], 0.0)
                nc.gpsimd.memset(h1T[p][k][:], 0.0)
        nc.gpsimd.memset(c0[:], 0.0)
        nc.gpsimd.memset(c1[:], 0.0)

        def gates(zp, c, htag):
            """z PSUM [128,1536] (layout i|f|o|g) -> h fp16 [128,H]."""
            sg = gp.tile([128, SIG], f16, tag="sg")
            nc.scalar.activation(sg[:], zp[:, 0:SIG], AF.Sigmoid)
            gt = gp.tile([128, H], f16, tag="gt")
            nc.scalar.activation(gt[:], zp[:, SIG:G4], AF.Tanh)
            t1 = gp.tile([128, H], f16, tag="t1")
            nc.vector.tensor_tensor(t1[:], sg[:, 0:H], gt[:], OP.mult)
            t2 = gp.tile([128, H], f32, tag="t2")
            nc.vector.tensor_tensor(t2[:], sg[:, H:2 * H], c[:], OP.mult)
            nc.vector.tensor_tensor(c[:], t1[:], t2[:], OP.add)
            tc_ = gp.tile([128, H], f16, tag="tc")
            nc.scalar.activation(tc_[:], c[:], AF.Tanh)
            h = gp.tile([128, H], f16, tag=f"h{htag}")
            nc.vector.tensor_tensor(h[:], sg[:, 2 * H:SIG], tc_[:], OP.mult)
            return h

        # ---------------- encoder ----------------
        flush_insts = []
        h1row = [None, None]   # h1 gate-output tiles by step parity
        flT2_v = flL_v = None

        xbufs = [None, None]
        xbufs[0] = xp.tile([DX + 1, SPI * BL], f16, tag="xbuf", name="xbuf")
        nc.sync.dma_start(xbufs[0][:].rearrange("p (s b) -> p s b", s=SPI),
                          d_xT[0:SPI].transpose([1, 0, 2]))

        def tr_h1_flush(t2):
            """h1(t2) -> h1T[t2%2]; fp8 side-copies into encres/flT2; flL row."""
            nonlocal flT2_v, flL_v
            fslot = t2 % SPI
            if fslot == 0:
                flT2 = fb.tile([128, SPI * BL], f8, tag="flT2")
                flT2_v = flT2[:].rearrange("p (b s) -> p b s", b=BL)
                flL = fb.tile([128, SPI * H], f8, tag="flL")
                flL_v = flL[:].rearrange("p (s d) -> p s d", s=SPI)
            h = h1row[t2 % 2]
            dst = h1T[t2 % 2]
            if TR_MODE == "dma":
                for k in range(3):
                    nc.sync.dma_start_transpose(dst[k][:], h[:, bass.ts(k, 128)])
                for k in range(2):
                    nc.vector.tensor_copy(encres_v[k][:, :, t2], dst[k][:])
                nc.vector.tensor_copy(flT2_v[:, :, fslot], dst[2][:])
            else:
                for k in range(3):
                    pt = ps.tile([128, 128], f16, tag="sc")
                    nc.tensor.transpose(pt[:], h[:, bass.ts(k, 128)], ident16[:])
                    nc.vector.tensor_copy(dst[k][:], pt[:])
                    if k < 2:
                        nc.vector.tensor_copy(encres_v[k][:, :, t2], pt[:])
                    else:
                        nc.vector.tensor_copy(flT2_v[:, :, fslot], pt[:])
            nc.vector.tensor_copy(flL_v[:, fslot, :], h[:])
            if fslot == SPI - 1:
                g0 = t2 - (SPI - 1)
                f1 = nc.sync.dma_start(d_encT2[:, :, g0:g0 + SPI], flT2_v[:, :, :])
                f2 = nc.sync.dma_start(
                    d_encL[g0:g0 + SPI].transpose([1, 0, 2]), flL_v[:, :, :])
                flush_insts.append(f1.ins)
                flush_insts.append(f2.ins)

        def z0_mms(t):
            zp = ps.tile([128, G4], f32, tag="z")
            xv = xbufs[(t // SPI) % 2][:, bass.ts(t % SPI, BL)]
            hp = h0T[(t - 1) % 2]
            for n in range(3):
                sl = slice(512 * n, 512 * (n + 1))
                nc.tensor.matmul(zp[:, sl], xv, wih0[:, sl], start=True, stop=False)
                for k in range(3):
                    nc.tensor.matmul(zp[:, sl], hp[k][:], whh0[k][:, sl],
                                     start=False, stop=(k == 2))
            return zp

        def z1_mms(t1_):
            zp = ps.tile([128, G4], f32, tag="z")
            hin = h0T[t1_ % 2]
            hrec = h1T[(t1_ - 1) % 2]
            for n in range(3):
                sl = slice(512 * n, 512 * (n + 1))
                nc.tensor.matmul(zp[:, sl], ones1[:], bias1[:, sl],
                                 start=True, stop=False)
                for k in range(3):
                    nc.tensor.matmul(zp[:, sl], hin[k][:], wih1[k][:, sl],
                                     start=False, stop=False)
                for k in range(3):
                    nc.tensor.matmul(zp[:, sl], hrec[k][:], whh1[k][:, sl],
                                     start=False, stop=(k == 2))
            return zp

        def tr_h0(t, h):
            dst = h0T[t % 2]
            if TR_MODE == "dma":
                for k in range(3):
                    nc.sync.dma_start_transpose(dst[k][:], h[:, bass.ts(k, 128)])
            else:
                for k in range(3):
                    pt = ps.tile([128, 128], f16, tag="sc")
                    nc.tensor.transpose(pt[:], h[:, bass.ts(k, 128)], ident16[:])
                    nc.scalar.copy(dst[k][:], pt[:])

        for t in range(L):
            if t % SPI == 0 and t // SPI + 1 < N_GROUPS:
                g = t // SPI + 1
                xbufs[g % 2] = xp.tile([DX + 1, SPI * BL], f16, tag="xbuf", name="xbuf")
                nc.sync.dma_start(
                    xbufs[g % 2][:].rearrange("p (s b) -> p s b", s=SPI),
                    d_xT[g * SPI:(g + 1) * SPI].transpose([1, 0, 2]))
            zp0 = z0_mms(t)
            if t >= 2:
                tr_h1_flush(t - 2)
            zp1 = z1_mms(t - 1) if t >= 1 else None
            h0 = gates(zp0, c0, "0")
            if t >= 1:
                h1row[(t - 1) % 2] = gates(zp1, c1, "1")
            tr_h0(t, h0)
        # epilogue: z1(L-1) + gates + final transposes/flush
        tr_h1_flush(L - 2)
        zp1 = z1_mms(L - 1)
        h1row[(L - 1) % 2] = gates(zp1, c1, "1")
        tr_h1_flush(L - 1)

        enc_es.close()

        # ---------------- decoder pools + weights ----------------
        wd = ctx.enter_context(tc.tile_pool(name="wdec", bufs=1))
        sp = ctx.enter_context(tc.tile_pool(name="stream", bufs=2))
        dg = ctx.enter_context(tc.tile_pool(name="decg", bufs=2))
        wa = [[wd.tile([128, 128], f16, tag=f"wa{k}{m}", name=f"wa{k}{m}")
               for m in range(3)] for k in range(3)]
        for k in range(3):
            for m in range(3):
                nc.sync.dma_start(wa[k][m][:], d_wa[k, m])
        dinw = [wd.tile([128, H], f16, tag=f"dinw{k}", name=f"dinw{k}")
                for k in range(3)]
        for k in range(3):
            nc.sync.dma_start(dinw[k][:], d_dinw[k])
        dinwt = wd.tile([6, H], f16, name="dinwt")
        nc.sync.dma_start(dinwt[:], d_dinwt)
        dwih = [wd.tile([128, G4], f16, tag=f"dwih{k}", name=f"dwih{k}")
                for k in range(3)]
        dwhh = [wd.tile([128, G4], f16, tag=f"dwhh{k}", name=f"dwhh{k}")
                for k in range(3)]
        for k in range(3):
            nc.sync.dma_start(dwih[k][:], d_dwih[k])
            nc.sync.dma_start(dwhh[k][:], d_dwhh[k])
        dbias = wd.tile([1, G4], f16, name="dbias")
        nc.sync.dma_start(dbias[:], d_dbias)
        outw = [wd.tile([128, 1], f16, tag=f"outw{k}", name=f"outw{k}")
                for k in range(3)]
        for k in range(3):
            nc.sync.dma_start(outw[k][:], d_outw[k])
        obrep = wd.tile([128, 1], f32, name="obrep")
        nc.sync.dma_start(obrep[:], d_outb[0:1, :].partition_broadcast(128))
        expb = wd.tile([128, 1], f32, name="expb")
        nc.gpsimd.memset(expb[:], -8.0)

        # join: decoder streams wait for all flushes via one funnel inst
        join = st.tile([1, 1], f32, name="join")
        jins = nc.gpsimd.memset(join[:], 0.0)
        for fi in flush_insts:
            add_dep_helper(jins.ins, fi, sync=True, reason="flush join")

        # ---------------- decoder ----------------
        cd = st.tile([128, H], f32, name="cd")
        nc.gpsimd.memset(cd[:], 0.0)
        tail = st.tile([6, 128], f16, name="tail")
        nc.gpsimd.memset(tail[:], 1.0)  # row5 stays ones
        nc.gpsimd.memset(tail[0:1, :], 0.0)  # y_prev = 0
        P4 = slice(0, 97, 32)  # partitions {0,32,64,96}

        def quad_dma_T2(qg):
            """encT2 d-chunk-2 stream for quad-group qg: b = 32j + 4qg + v."""
            tl_ = sp.tile([128, 16 * L], f8, tag="sT2")
            v4 = tl_[:].rearrange("p (j v l) -> p j v l", j=4, v=4)
            dma = nc.sync.dma_start(
                v4, d_encT2[:, :, :].rearrange("p (j u) l -> p j u l", j=4)
                [:, :, 4 * qg:4 * qg + 4, :])
            add_dep_helper(dma.ins, jins.ins, sync=True, reason="enc stream")
            return v4

        def quad_dma_L(qg):
            """encL stream tiles per l-chunk for quad-group qg."""
            out = []
            for ci, (l0, kl) in enumerate(L_CHUNKS):
                tl_ = sp.tile([128, 16 * H], f8, tag=f"sL{ci}")
                v4 = tl_[:].rearrange("p (j v d) -> p j v d", j=4, v=4)
                dma = nc.sync.dma_start(
                    v4[0:kl],
                    d_encL[l0:l0 + kl]
                    .rearrange("p (j u) d -> p j u d", j=4)[:, :, 4 * qg:4 * qg + 4, :])
                add_dep_helper(dma.ins, jins.ins, sync=True, reason="enc stream")
                out.append(v4)
            return out

        def dec_step(t):
            nc.sync.dma_start(tail[1:5, :], d_featT[bass.ds(t, 1)].squeeze(0))
            rdT = h1T[(L - 1) % 2] if t == 0 else hdT[(t - 1) % 2]
            wrT = hdT[t % 2]
            # g = Wa^T h -> gT8 [128 d', (k,b)] fp8
            gps = ps.tile([128, H], f32, tag="z")
            for m in range(3):
                for k in range(3):
                    nc.tensor.matmul(gps[:, bass.ts(m, 128)], wa[k][m][:],
                                     rdT[k][:], start=(k == 0), stop=(k == 2))
            gT8 = dg.tile([128, H], f8, tag="gT8")
            nc.vector.tensor_copy(gT8[:], gps[:])

            # ---- scores: col-tiled quads; evac rows -> fan-in -> softmax ----
            sc_all = dg.tile([128, L], f16, tag="sc_all", bufs=1)
            sT2 = None
            for qg in range(8):
                nT2 = quad_dma_T2(qg) if qg == 0 else sT2_next
                sT2 = nT2
                if qg < 7:
                    sT2_next = quad_dma_T2(qg + 1)
                scbuf = dg.tile([128, 4 * L], f16, tag="scbuf")
                scb_v = scbuf[:].rearrange("p (v l) -> p v l", v=4)
                for v in range(4):
                    r = 4 * qg + v
                    scps = ps.tile([128, 512], f32, tag="sc")
                    for j in range(4):
                        b = 32 * j + r
                        for k in range(3):
                            rhs = (encres_v[k][:, b, :] if k < 2
                                   else sT2[:, j, v, :])
                            nc.tensor.matmul(
                                scps[32 * j:32 * j + 1, 0:L],
                                gT8[:, 128 * k + b:128 * k + b + 1], rhs,
                                start=(k == 0), stop=(k == 2),
                                tile_position=(0, 32 * j))
                    for j in range(4):
                        pr_ = slice(32 * j, 32 * j + 1)
                        if j < 2:
                            nc.scalar.copy(scb_v[pr_, v, :], scps[pr_, 0:L])
                        else:
                            nc.vector.tensor_copy(scb_v[pr_, v, :],
                                                  scps[pr_, 0:L])
                for j in range(4):
                    nc.sync.dma_start(
                        sc_all[32 * j + 4 * qg:32 * j + 4 * qg + 4, :],
                        scbuf[32 * j:32 * j + 1, :])
            # softmax over the contiguous [b, l] tile (shift -8 for fp16 range)
            eall = dg.tile([128, L], f16, tag="eall", bufs=1)
            sums = dg.tile([128, 1], f32, tag="sums")
            nc.scalar.activation(eall[:], sc_all[:], AF.Exp, bias=expb[:, 0:1],
                                 accum_out=sums[:])
            rcp = dg.tile([128, 1], f32, tag="rcp")
            nc.vector.reciprocal(rcp[:], sums[:])
            arow = dg.tile([128, 128 * N_LC], f16, tag="arow", bufs=1)
            if 128 * N_LC > L:
                nc.gpsimd.memset(arow[:, L:128 * N_LC], 0.0)
            nc.vector.tensor_scalar_mul(arow[:, 0:L], eall[:], rcp[:])
            alphaT8 = []
            for ci, (l0, kl) in enumerate(L_CHUNKS):
                a16 = dg.tile([128, 128], f16, tag=f"aT16{ci}", name=f"aT16{ci}")
                nc.sync.dma_start_transpose(
                    a16[:], arow[:, 128 * ci:128 * ci + 128])
                a8 = dg.tile([128, 128], f8, tag=f"aT8{ci}", name=f"aT8{ci}")
                # alpha is diffuse (~1/L): scale x128 out of fp8-subnormal range
                nc.vector.tensor_scalar_mul(a8[:], a16[:], 128.0)
                alphaT8.append(a8)

            # ---- ctx: col-tiled quads; evac (x 1/128) -> fan-in -> transpose ----
            ctx_all = dg.tile([128, H], f16, tag="ctx_all", bufs=1)
            for qg in range(8):
                nL = quad_dma_L(qg) if qg == 0 else sL_next
                sL = nL
                if qg < 7:
                    sL_next = quad_dma_L(qg + 1)
                cxbuf = dg.tile([128, 4 * H], f16, tag="cxbuf")
                cxb_v = cxbuf[:].rearrange("p (v d) -> p v d", v=4)
                for v in range(4):
                    r = 4 * qg + v
                    cxps = ps.tile([128, 512], f32, tag="sc")
                    for j in range(4):
                        b = 32 * j + r
                        for ci, (l0, kl) in enumerate(L_CHUNKS):
                            nc.tensor.matmul(
                                cxps[32 * j:32 * j + 1, 0:H],
                                alphaT8[ci][0:kl, b:b + 1],
                                sL[ci][0:kl, j, v, :],
                                start=(ci == 0), stop=(ci == N_LC - 1),
                                tile_position=(0, 32 * j))
                    for j in range(4):
                        pr_ = slice(32 * j, 32 * j + 1)
                        if j < 2:
                            nc.scalar.activation(cxb_v[pr_, v, :],
                                                 cxps[pr_, 0:H], AF.Copy,
                                                 scale=1.0 / 128.0)
                        else:
                            nc.vector.tensor_scalar_mul(cxb_v[pr_, v, :],
                                                        cxps[pr_, 0:H],
                                                        1.0 / 128.0)
                for j in range(4):
                    nc.sync.dma_start(
                        ctx_all[32 * j + 4 * qg:32 * j + 4 * qg + 4, :],
                        cxbuf[32 * j:32 * j + 1, :])
            ctxT = []
            for k in range(3):
                c16 = dg.tile([128, 128], f16, tag=f"cT16{k}", name=f"cT16{k}")
                nc.sync.dma_start_transpose(
                    c16[:], ctx_all[:, 128 * k:128 * k + 128])
                ctxT.append(c16)

            # ---- dec_in ----
            dps = ps.tile([128, H], f32, tag="z")
            for k in range(3):
                nc.tensor.matmul(dps[:], ctxT[k][:], dinw[k][:],
                                 start=(k == 0), stop=False)
            nc.tensor.matmul(dps[:], tail[:], dinwt[:], start=False, stop=True)
            din_a = dg.tile([128, H], f16, tag="din_a")
            nc.scalar.activation(din_a[:], dps[:], AF.Relu)
            daT = []
            for k in range(3):
                pt = ps.tile([128, 128], f16, tag="sc")
                nc.tensor.transpose(pt[:], din_a[:, bass.ts(k, 128)], ident16[:])
                dt_ = dg.tile([128, 128], f16, tag=f"daT{k}", name=f"daT{k}")
                nc.scalar.copy(dt_[:], pt[:])
                daT.append(dt_)
            # ---- decoder LSTM ----
            zp = ps.tile([128, G4], f32, tag="z")
            for n in range(3):
                sl = slice(512 * n, 512 * (n + 1))
                nc.tensor.matmul(zp[:, sl], ones1[:], dbias[:, sl],
                                 start=True, stop=False)
                for k in range(3):
                    nc.tensor.matmul(zp[:, sl], daT[k][:], dwih[k][:, sl],
                                     start=False, stop=False)
                for k in range(3):
                    nc.tensor.matmul(zp[:, sl], rdT[k][:], dwhh[k][:, sl],
                                     start=False, stop=(k == 2))
            hd = gates(zp, cd, "d")
            for k in range(3):
                pt = ps.tile([128, 128], f16, tag="sc")
                nc.tensor.transpose(pt[:], hd[:, bass.ts(k, 128)], ident16[:])
                nc.scalar.copy(wrT[k][:], pt[:])
            # ---- output head ----
            yps = ps.tile([128, 512], f32, tag="sc")
            for k in range(3):
                nc.tensor.matmul(yps[:, 0:1], wrT[k][:], outw[k][:],
                                 start=(k == 0), stop=(k == 2))
            ybs = dg.tile([128, 1], f32, tag="ybs")
            nc.scalar.activation(ybs[:], yps[:, 0:1], AF.Identity,
                                 bias=obrep[:, 0:1])
            nc.sync.dma_start(d_y[:, bass.ds(t, 1)], ybs[:])
            ytp = ps.tile([128, 512], f32, tag="sc")
            nc.tensor.transpose(ytp[0:1, 0:128], ybs[:], ident[:])
            nc.scalar.copy(tail[0:1, :], ytp[0:1, 0:128])

        for t in range(HZ):
            dec_step(t)

    nc.compile()
    return nc


def _perm_gates_rows(w):
    """Reorder LSTM gate rows from [i,f,g,o] to [i,f,o,g]. w: [4H, ...] or [4H]."""
    return np.concatenate([w[0:H], w[H:2 * H], w[3 * H:4 * H], w[2 * H:3 * H]], 0)


def _prep(inputs):
    """Host-side packing of all weights/inputs into device layouts."""
    g = {k: np.asarray(v, np.float32) for k, v in inputs.items()}
    h16 = lambda a: np.ascontiguousarray(a, dtype=np.float16)
    P = _perm_gates_rows
    pr = {}
    pr["wih0"] = h16(np.concatenate(
        [P(g["enc_Wih0"]).T, P(g["enc_bih0"] + g["enc_bhh0"])[None, :]], 0))
    pr["whh0"] = h16(P(g["enc_Whh0"]).T.reshape(3, 128, G4))
    pr["wih1"] = h16(P(g["enc_Wih1"]).T.reshape(3, 128, G4))
    pr["whh1"] = h16(P(g["enc_Whh1"]).T.reshape(3, 128, G4))
    pr["bias1"] = h16(P(g["enc_bih1"] + g["enc_bhh1"])[None, :])
    wa = g["Wa"]  # [dec_h, enc_h]
    pr["wa"] = h16(wa.reshape(3, 128, 3, 128).transpose(0, 2, 1, 3))
    W = g["dec_in_W"]; bvec = g["dec_in_b"]
    Wp = np.concatenate([W[:, 5:389], W[:, 0:1], W[:, 1:5], bvec[:, None]], 1)
    WpT = Wp.T  # [390, 384]
    pr["dinw"] = h16(WpT[:384].reshape(3, 128, H))
    pr["dinwt"] = h16(WpT[384:390])
    pr["dwih"] = h16(P(g["dec_Wih"]).T.reshape(3, 128, G4))
    pr["dwhh"] = h16(P(g["dec_Whh"]).T.reshape(3, 128, G4))
    pr["dbias"] = h16(P(g["dec_bih"] + g["dec_bhh"])[None, :])
    pr["outw"] = h16(g["out_W"].T.reshape(3, 128, 1))
    pr["outb"] = np.ascontiguousarray(g["out_b"].reshape(1, 1), np.float32)
    return g, pr


def _core_inputs(g, pr, c):
    sl = slice(c * BL, (c + 1) * BL)
    x = g["x"][sl]                     # [128, L, 8]
    xe = np.concatenate([x, np.ones((BL, L, 1), np.float32)], 2)
    m = dict(pr)
    m["xT"] = np.ascontiguousarray(xe.transpose(1, 2, 0), np.float16)
    m["featT"] = np.ascontiguousarray(
        g["future_feats"][sl].transpose(1, 2, 0), np.float16)
    return m


def kernel(**inputs):
    if "nc" not in _cache:
        _cache["nc"] = _build()
    nc = _cache["nc"]
    g, pr = _prep(inputs)
    in_maps = [_core_inputs(g, pr, c) for c in range(NC)]
    res = bass_utils.run_bass_kernel_spmd(nc, in_maps, core_ids=list(range(NC)))
    out = np.concatenate([res.results[c]["y"] for c in range(NC)], 0)
    return np.ascontiguousarray(out[:, :, None], np.float32)


# revision 10
# speedup vs baseline: 1.0676x; 1.0676x over previous
"""Trainium2 Bass kernel v2 for nn_AttnSeq2Seq (2-layer LSTM encoder + attention decoder).

Sharding: pure data parallelism - batch 1024 = 8 x 128, weights replicated.

Encoder (L steps, 2 fused layers, software-pipelined with a 1-step skew):
  per step t the PE queue is [z0(t) | tr-h1(t-2) | z1(t-1) | tr-h0(t)] so the
  gate latency (ACT+DVE) of each layer hides under the other layer's matmuls.
  Gate columns are permuted to [i|f|o|g]: one Sigmoid over 1152 cols + one Tanh.
  enc_out (h1) is written to fp8: d-chunks 0,1 stay resident in SBUF as
  [d, b, l]; d-chunk 2 and the [l, b, d] layout are spilled to DRAM.
Decoder (HZ steps): scores via 4-way column-tiled M=1 matmuls (4 examples run
  concurrently in separate 32-column PE groups) against fp8 enc, softmax
  normalized at the source (ACT accum_out + per-partition reciprocal scale),
  alpha/ctx transposed via DRAM-round-trip dma_start_transpose, ctx via
  col-tiled M=1 matmuls against the fp8 [l, b, d] stream. No DVE inner loops,
  no per-example partition broadcasts.
"""
import os
import numpy as np
from contextlib import ExitStack

import concourse.bass as bass
import concourse.tile as tile
from concourse import bacc, mybir, bass_utils, masks
from concourse.tile import add_dep_helper

f32 = mybir.dt.float32
f16 = mybir.dt.float16
f8 = mybir.dt.float8e4
AF = mybir.ActivationFunctionType
OP = mybir.AluOpType

B, DX, H = 1024, 8, 384
L = int(os.environ.get("K_L", "336"))
HZ = int(os.environ.get("K_HZ", "18"))
NC = 8
BL = B // NC          # 128 per core
G4 = 4 * H            # 1536
SIG = 3 * H           # sigmoid span after [i|f|o|g] permutation
SPI = 16              # encoder steps per x/flush group
N_GROUPS = L // SPI
assert L % SPI == 0
TR_MODE = os.environ.get("TR_MODE", "pe")  # encoder h transposes: 'pe' | 'dma'
N_LC = (L + 127) // 128
L_CHUNKS = [(c * 128, min(128, L - 128 * c)) for c in range(N_LC)]

_cache = {}


def _build():
    nc = bacc.Bacc("TRN2", target_bir_lowering=False, debug=False)

    # ---------------- DRAM I/O ----------------
    d_xT = nc.dram_tensor("xT", [L, DX + 1, BL], f16, kind="ExternalInput").ap()
    d_wih0 = nc.dram_tensor("wih0", [DX + 1, G4], f16, kind="ExternalInput").ap()
    d_whh0 = nc.dram_tensor("whh0", [3, 128, G4], f16, kind="ExternalInput").ap()
    d_wih1 = nc.dram_tensor("wih1", [3, 128, G4], f16, kind="ExternalInput").ap()
    d_whh1 = nc.dram_tensor("whh1", [3, 128, G4], f16, kind="ExternalInput").ap()
    d_bias1 = nc.dram_tensor("bias1", [1, G4], f16, kind="ExternalInput").ap()
    d_wa = nc.dram_tensor("wa", [3, 3, 128, 128], f16, kind="ExternalInput").ap()
    d_dinw = nc.dram_tensor("dinw", [3, 128, H], f16, kind="ExternalInput").ap()
    d_dinwt = nc.dram_tensor("dinwt", [6, H], f16, kind="ExternalInput").ap()
    d_dwih = nc.dram_tensor("dwih", [3, 128, G4], f16, kind="ExternalInput").ap()
    d_dwhh = nc.dram_tensor("dwhh", [3, 128, G4], f16, kind="ExternalInput").ap()
    d_dbias = nc.dram_tensor("dbias", [1, G4], f16, kind="ExternalInput").ap()
    d_outw = nc.dram_tensor("outw", [3, 128, 1], f16, kind="ExternalInput").ap()
    d_outb = nc.dram_tensor("outb", [1, 1], f32, kind="ExternalInput").ap()
    d_featT = nc.dram_tensor("featT", [HZ, 4, BL], f16, kind="ExternalInput").ap()
    d_y = nc.dram_tensor("y", [BL, HZ], f32, kind="ExternalOutput").ap()
    # device-written scratch
    d_encT2 = nc.dram_tensor("encT2", [128, BL, L], f8, kind="ExternalOutput").ap()
    d_encL = nc.dram_tensor("encL", [L, BL, H], f8, kind="ExternalOutput").ap()

    with tile.TileContext(nc) as tc, ExitStack() as ctx:
        wp = ctx.enter_context(tc.tile_pool(name="wpers", bufs=1))
        st = ctx.enter_context(tc.tile_pool(name="state", bufs=1))
        gp = ctx.enter_context(tc.tile_pool(name="gates", bufs=2))
        wd = ctx.enter_context(tc.tile_pool(name="wdec", bufs=1))
        enc_es = ExitStack()
        ps = enc_es.enter_context(tc.tile_pool(name="psum", bufs=2, space="PSUM"))
        xp = enc_es.enter_context(tc.tile_pool(name="xin", bufs=2))
        fb = enc_es.enter_context(tc.tile_pool(name="flush", bufs=2))
        we = enc_es.enter_context(tc.tile_pool(name="wenc", bufs=1))

        # ---------------- persistent consts ----------------
        ident = wp.tile([128, 128], f32)
        masks.make_identity(nc, ident[:])
        ident16 = wp.tile([128, 128], f16)
        masks.make_identity(nc, ident16[:])
        ones1 = wp.tile([1, 128], f16, name="ones1")
        nc.gpsimd.memset(ones1[:], 1.0)
        # resident fp8 enc_out d-chunks 0,1: [128 d, BL, L]
        encres = [wp.tile([128, BL * L], f8, tag=f"encres{k}", name=f"encres{k}") for k in range(2)]
        encres_v = [t[:].rearrange("p (b l) -> p b l", b=BL) for t in encres]

        # ---------------- encoder-only weights (freed before decoder) --------
        wih0 = we.tile([DX + 1, G4], f16, name="wih0")
        nc.sync.dma_start(wih0[:], d_wih0)
        whh0 = [we.tile([128, G4], f16, tag=f"whh0{k}", name=f"whh0{k}") for k in range(3)]
        wih1 = [we.tile([128, G4], f16, tag=f"wih1{k}", name=f"wih1{k}") for k in range(3)]
        whh1 = [we.tile([128, G4], f16, tag=f"whh1{k}", name=f"whh1{k}") for k in range(3)]
        for k in range(3):
            nc.sync.dma_start(whh0[k][:], d_whh0[k])
            nc.sync.dma_start(wih1[k][:], d_wih1[k])
            nc.sync.dma_start(whh1[k][:], d_whh1[k])
        bias1 = we.tile([1, G4], f16, name="bias1")
        nc.sync.dma_start(bias1[:], d_bias1)

        # decoder weights loaded up-front (overlap with encoder)
        wa = [[wd.tile([128, 128], f16, tag=f"wa{k}{m}", name=f"wa{k}{m}")
               for m in range(3)] for k in range(3)]
        for k in range(3):
            for m in range(3):
                nc.sync.dma_start(wa[k][m][:], d_wa[k, m])
        dinw = [wd.tile([128, H], f16, tag=f"dinw{k}", name=f"dinw{k}")
                for k in range(3)]
        for k in range(3):
            nc.sync.dma_start(dinw[k][:], d_dinw[k])
        dinwt = wd.tile([6, H], f16, name="dinwt")
        nc.sync.dma_start(dinwt[:], d_dinwt)
        dwih = [wd.tile([128, G4], f16, tag=f"dwih{k}", name=f"dwih{k}")
                for k in range(3)]
        dwhh = [wd.tile([128, G4], f16, tag=f"dwhh{k}", name=f"dwhh{k}")
                for k in range(3)]
        for k in range(3):
            nc.sync.dma_start(dwih[k][:], d_dwih[k])
            nc.sync.dma_start(dwhh[k][:], d_dwhh[k])
        dbias = wd.tile([1, G4], f16, name="dbias")
        nc.sync.dma_start(dbias[:], d_dbias)
        outw = [wd.tile([128, 1], f16, tag=f"outw{k}", name=f"outw{k}")
                for k in range(3)]
        for k in range(3):
            nc.sync.dma_start(outw[k][:], d_outw[k])
        obrep = wd.tile([128, 1], f32, name="obrep")
        nc.sync.dma_start(obrep[:], d_outb[0:1, :].partition_broadcast(128))
        expb = wd.tile([128, 1], f32, name="expb")
        nc.gpsimd.memset(expb[:], -8.0)

        # ---------------- state ----------------
        h0T = [[st.tile([128, 128], f16, tag=f"h0T{p}{k}", name=f"h0T{p}{k}") for k in range(3)]
               for p in range(2)]
        h1T = [[st.tile([128, 128], f16, tag=f"h1T{p}{k}", name=f"h1T{p}{k}") for k in range(3)]
               for p in range(2)]
        hdT = [[st.tile([128, 128], f16, tag=f"hdT{p}{k}", name=f"hdT{p}{k}") for k in range(3)]
               for p in range(2)]
        c0 = st.tile([128, H], f16, name="c0")
        c1 = st.tile([128, H], f16, name="c1")
        for p in range(2):
            for k in range(3):
                nc.gpsimd.memset(h0T[p][k][:], 0.0)
                nc.gpsimd.memset(h1T[p][k][:], 0.0)
        nc.gpsimd.memset(c0[:], 0.0)
        nc.gpsimd.memset(c1[:], 0.0)

        def gates(zp, c, htag):
            """z PSUM [128,1536] (layout i|f|o|g) -> h fp16 [128,H]."""
            sg = gp.tile([128, SIG], f16, tag="sg")
            nc.scalar.activation(sg[:], zp[:, 0:SIG], AF.Sigmoid)
            gt = gp.tile([128, H], f16, tag="gt")
            nc.scalar.activation(gt[:], zp[:, SIG:G4], AF.Tanh)
            t1 = gp.tile([128, H], f16, tag="t1")
            nc.vector.tensor_tensor(t1[:], sg[:, 0:H], gt[:], OP.mult)
            t2 = gp.tile([128, H], f16, tag="t2")
            nc.vector.tensor_tensor(t2[:], sg[:, H:2 * H], c[:], OP.mult)
            nc.vector.tensor_tensor(c[:], t1[:], t2[:], OP.add)
            tc_ = gp.tile([128, H], f16, tag="tc")
            nc.scalar.activation(tc_[:], c[:], AF.Tanh)
            h = gp.tile([128, H], f16, tag=f"h{htag}")
            nc.vector.tensor_tensor(h[:], sg[:, 2 * H:SIG], tc_[:], OP.mult)
            return h

        # ---------------- encoder ----------------
        flush_insts = []
        h1row = [None, None]   # h1 gate-output tiles by step parity
        flT2_v = flL_v = None

        xbufs = [None, None]
        xbufs[0] = xp.tile([DX + 1, SPI * BL], f16, tag="xbuf", name="xbuf")
        nc.sync.dma_start(xbufs[0][:].rearrange("p (s b) -> p s b", s=SPI),
                          d_xT[0:SPI].transpose([1, 0, 2]))

        def tr_h1_flush(t2):
            """h1(t2) -> h1T[t2%2]; fp8 side-copies into encres/flT2; flL row."""
            nonlocal flT2_v, flL_v
            fslot = t2 % SPI
            if fslot == 0:
                flT2 = fb.tile([128, SPI * BL], f8, tag="flT2")
                flT2_v = flT2[:].rearrange("p (b s) -> p b s", b=BL)
                flL = fb.tile([128, SPI * H], f8, tag="flL")
                flL_v = flL[:].rearrange("p (s d) -> p s d", s=SPI)
            h = h1row[t2 % 2]
            dst = h1T[t2 % 2]
            if TR_MODE == "dma":
                for k in range(3):
                    nc.sync.dma_start_transpose(dst[k][:], h[:, bass.ts(k, 128)])
                for k in range(2):
                    nc.vector.tensor_copy(encres_v[k][:, :, t2], dst[k][:])
                nc.vector.tensor_copy(flT2_v[:, :, fslot], dst[2][:])
            else:
                for k in range(3):
                    pt = ps.tile([128, 128], f16, tag="sc")
                    nc.tensor.transpose(pt[:], h[:, bass.ts(k, 128)], ident16[:])
                    nc.vector.tensor_copy(dst[k][:], pt[:])
                    if k < 2:
                        nc.vector.tensor_copy(encres_v[k][:, :, t2], pt[:])
                    else:
                        nc.vector.tensor_copy(flT2_v[:, :, fslot], pt[:])
            nc.gpsimd.tensor_copy(flL_v[:, fslot, :], h[:])
            if fslot == SPI - 1:
                g0 = t2 - (SPI - 1)
                f1 = nc.sync.dma_start(d_encT2[:, :, g0:g0 + SPI], flT2_v[:, :, :])
                f2 = nc.sync.dma_start(
                    d_encL[g0:g0 + SPI].transpose([1, 0, 2]), flL_v[:, :, :])
                flush_insts.append(f1.ins)
                flush_insts.append(f2.ins)

        def z0_mms(t):
            zp = ps.tile([128, G4], f32, tag="z")
            xv = xbufs[(t // SPI) % 2][:, bass.ts(t % SPI, BL)]
            hp = h0T[(t - 1) % 2]
            for n in range(3):
                sl = slice(512 * n, 512 * (n + 1))
                nc.tensor.matmul(zp[:, sl], xv, wih0[:, sl], start=True, stop=False)
            for n in range(3):
                sl = slice(512 * n, 512 * (n + 1))
                for k in range(3):
                    nc.tensor.matmul(zp[:, sl], hp[k][:], whh0[k][:, sl],
                                     start=False, stop=(k == 2))
            return zp

        def z1_mms(t1_):
            zp = ps.tile([128, G4], f32, tag="z")
            hin = h0T[t1_ % 2]
            hrec = h1T[(t1_ - 1) % 2]
            for n in range(3):
                sl = slice(512 * n, 512 * (n + 1))
                nc.tensor.matmul(zp[:, sl], ones1[:], bias1[:, sl],
                                 start=True, stop=False)
                for k in range(3):
                    nc.tensor.matmul(zp[:, sl], hin[k][:], wih1[k][:, sl],
                                     start=False, stop=False)
                for k in range(3):
                    nc.tensor.matmul(zp[:, sl], hrec[k][:], whh1[k][:, sl],
                                     start=False, stop=(k == 2))
            return zp

        def tr_h0(t, h):
            dst = h0T[t % 2]
            if TR_MODE == "dma":
                for k in range(3):
                    nc.sync.dma_start_transpose(dst[k][:], h[:, bass.ts(k, 128)])
            else:
                for k in range(3):
                    pt = ps.tile([128, 128], f16, tag="sc")
                    nc.tensor.transpose(pt[:], h[:, bass.ts(k, 128)], ident16[:])
                    nc.scalar.copy(dst[k][:], pt[:])

        for t in range(L):
            if t % SPI == 0 and t // SPI + 1 < N_GROUPS:
                g = t // SPI + 1
                xbufs[g % 2] = xp.tile([DX + 1, SPI * BL], f16, tag="xbuf", name="xbuf")
                nc.sync.dma_start(
                    xbufs[g % 2][:].rearrange("p (s b) -> p s b", s=SPI),
                    d_xT[g * SPI:(g + 1) * SPI].transpose([1, 0, 2]))
            zp0 = z0_mms(t)
            if t >= 2:
                tr_h1_flush(t - 2)
            zp1 = z1_mms(t - 1) if t >= 1 else None
            h0 = gates(zp0, c0, "0")
            if t >= 1:
                h1row[(t - 1) % 2] = gates(zp1, c1, "1")
            tr_h0(t, h0)
        # epilogue: z1(L-1) + gates + final transposes/flush
        tr_h1_flush(L - 2)
        zp1 = z1_mms(L - 1)
        h1row[(L - 1) % 2] = gates(zp1, c1, "1")
        tr_h1_flush(L - 1)

        enc_es.close()

        # ---------------- decoder pools ----------------
        p2 = ctx.enter_context(tc.tile_pool(name="psum2", bufs=1, space="PSUM"))
        sp = ctx.enter_context(tc.tile_pool(name="stream", bufs=2))
        dg = ctx.enter_context(tc.tile_pool(name="decg", bufs=2))

        # join: decoder streams wait for all flushes via one funnel inst
        join = st.tile([1, 1], f32, name="join")
        jins = nc.gpsimd.memset(join[:], 0.0)
        for fi in flush_insts:
            add_dep_helper(jins.ins, fi, sync=True, reason="flush join")

        # ---------------- decoder ----------------
        cd = st.tile([128, H], f16, name="cd")
        nc.gpsimd.memset(cd[:], 0.0)
        tail = st.tile([6, 128], f16, name="tail")
        nc.gpsimd.memset(tail[:], 1.0)  # row5 stays ones
        nc.gpsimd.memset(tail[0:1, :], 0.0)  # y_prev = 0
        P4 = slice(0, 97, 32)  # partitions {0,32,64,96}

        def quad_dma_T2(qg):
            """encT2 d-chunk-2 stream for quad-group qg: b = 32j + 4qg + v."""
            tl_ = sp.tile([128, 16 * L], f8, tag="sT2")
            v4 = tl_[:].rearrange("p (j v l) -> p j v l", j=4, v=4)
            dma = nc.sync.dma_start(
                v4, d_encT2[:, :, :].rearrange("p (j u) l -> p j u l", j=4)
                [:, :, 4 * qg:4 * qg + 4, :])
            add_dep_helper(dma.ins, jins.ins, sync=True, reason="enc stream")
            return v4

        def quad_dma_L(qg):
            """encL stream tiles per l-chunk for quad-group qg."""
            out = []
            for ci, (l0, kl) in enumerate(L_CHUNKS):
                tl_ = sp.tile([128, 16 * H], f8, tag=f"sL{ci}")
                v4 = tl_[:].rearrange("p (j v d) -> p j v d", j=4, v=4)
                dma = nc.sync.dma_start(
                    v4[0:kl],
                    d_encL[l0:l0 + kl]
                    .rearrange("p (j u) d -> p j u d", j=4)[:, :, 4 * qg:4 * qg + 4, :])
                add_dep_helper(dma.ins, jins.ins, sync=True, reason="enc stream")
                out.append(v4)
            return out

        def dec_step(t):
            nc.sync.dma_start(tail[1:5, :], d_featT[bass.ds(t, 1)].squeeze(0))
            rdT = h1T[(L - 1) % 2] if t == 0 else hdT[(t - 1) % 2]
            wrT = hdT[t % 2]
            # g = Wa^T h -> gT8 [128 d', (k,b)] fp8
            gps = p2.tile([128, G4], f32, tag="dz", bufs=1, name="gps")[:, 0:H]
            for m in range(3):
                for k in range(3):
                    nc.tensor.matmul(gps[:, bass.ts(m, 128)], wa[k][m][:],
                                     rdT[k][:], start=(k == 0), stop=(k == 2))
            gT8 = dg.tile([128, H], f8, tag="gT8")
            nc.vector.tensor_copy(gT8[:], gps[:])
            sL_pre = quad_dma_L(0)

            # ---- scores: col-tiled quads; evac rows -> fan-in -> softmax ----
            sc_all = dg.tile([128, L], f16, tag="sc_all", bufs=1)
            sT2 = None
            for qg in range(8):
                nT2 = quad_dma_T2(qg) if qg == 0 else sT2_next
                sT2 = nT2
                if qg < 7:
                    sT2_next = quad_dma_T2(qg + 1)
                scbuf = dg.tile([128, 4 * L], f16, tag="scbuf")
                scb_v = scbuf[:].rearrange("p (v l) -> p v l", v=4)
                for hv in range(2):
                    scps = p2.tile([128, 1024], f32, tag="sc2", bufs=2,
                                   name="scps")
                    sv2 = scps[:].rearrange("p (i c) -> p i c", i=2)
                    for u in range(2):
                        v = 2 * hv + u
                        r = 4 * qg + v
                        for j in range(4):
                            b = 32 * j + r
                            for k in range(3):
                                rhs = (encres_v[k][:, b, :] if k < 2
                                       else sT2[:, j, v, :])
                                nc.tensor.matmul(
                                    sv2[32 * j:32 * j + 1, u, 0:L],
                                    gT8[:, 128 * k + b:128 * k + b + 1], rhs,
                                    start=(k == 0), stop=(k == 2),
                                    tile_position=(0, 32 * j))
                    for j in range(4):
                        pr_ = slice(32 * j, 32 * j + 1)
                        if j < 2:
                            nc.scalar.copy(scb_v[pr_, 2 * hv:2 * hv + 2, :],
                                           sv2[pr_, :, 0:L])
                        else:
                            nc.vector.tensor_copy(
                                scb_v[pr_, 2 * hv:2 * hv + 2, :],
                                sv2[pr_, :, 0:L])
                for j in range(4):
                    nc.sync.dma_start(
                        sc_all[32 * j + 4 * qg:32 * j + 4 * qg + 4, :],
                        scbuf[32 * j:32 * j + 1, :])
            # softmax over the contiguous [b, l] tile (shift -8 for fp16 range)
            eall = dg.tile([128, L], f16, tag="eall", bufs=1)
            sums = dg.tile([128, 1], f32, tag="sums")
            nc.scalar.activation(eall[:], sc_all[:], AF.Exp, bias=expb[:, 0:1],
                                 accum_out=sums[:])
            rcp = dg.tile([128, 1], f32, tag="rcp")
            nc.vector.reciprocal(rcp[:], sums[:])
            arow = dg.tile([128, 128 * N_LC], f16, tag="arow", bufs=1)
            if 128 * N_LC > L:
                nc.gpsimd.memset(arow[:, L:128 * N_LC], 0.0)
            nc.vector.tensor_scalar_mul(arow[:, 0:L], eall[:], rcp[:])
            alphaT8 = []
            for ci, (l0, kl) in enumerate(L_CHUNKS):
                a16 = dg.tile([128, 128], f16, tag=f"aT16{ci}", name=f"aT16{ci}")
                nc.sync.dma_start_transpose(
                    a16[:], arow[:, 128 * ci:128 * ci + 128])
                a8 = dg.tile([128, 128], f8, tag=f"aT8{ci}", name=f"aT8{ci}")
                # alpha is diffuse (~1/L): scale x128 out of fp8-subnormal range
                nc.vector.tensor_scalar_mul(a8[:], a16[:], 128.0)
                alphaT8.append(a8)

            # ---- ctx: col-tiled quads; evac (x 1/128) -> fan-in -> transpose ----
            ctx_all = dg.tile([128, H], f16, tag="ctx_all", bufs=1)
            for qg in range(8):
                sL = sL_pre if qg == 0 else sL_next
                if qg < 7:
                    sL_next = quad_dma_L(qg + 1)
                cxbuf = dg.tile([128, 4 * H], f16, tag="cxbuf")
                cxb_v = cxbuf[:].rearrange("p (v d) -> p v d", v=4)
                for hv in range(2):
                    cxps = p2.tile([128, 1024], f32, tag="sc2", bufs=2,
                                   name="cxps")
                    cv2 = cxps[:].rearrange("p (i c) -> p i c", i=2)
                    for u in range(2):
                        v = 2 * hv + u
                        r = 4 * qg + v
                        for j in range(4):
                            b = 32 * j + r
                            for ci, (l0, kl) in enumerate(L_CHUNKS):
                                nc.tensor.matmul(
                                    cv2[32 * j:32 * j + 1, u, 0:H],
                                    alphaT8[ci][0:kl, b:b + 1],
                                    sL[ci][0:kl, j, v, :],
                                    start=(ci == 0), stop=(ci == N_LC - 1),
                                    tile_position=(0, 32 * j))
                    for j in range(4):
                        pr_ = slice(32 * j, 32 * j + 1)
                        if j < 2:
                            nc.scalar.activation(cxb_v[pr_, 2 * hv:2 * hv + 2, :],
                                                 cv2[pr_, :, 0:H], AF.Copy,
                                                 scale=1.0 / 128.0)
                        else:
                            nc.vector.tensor_scalar_mul(
                                cxb_v[pr_, 2 * hv:2 * hv + 2, :],
                                cv2[pr_, :, 0:H], 1.0 / 128.0)
                for j in range(4):
                    nc.sync.dma_start(
                        ctx_all[32 * j + 4 * qg:32 * j + 4 * qg + 4, :],
                        cxbuf[32 * j:32 * j + 1, :])
            ctxT = []
            for k in range(3):
                c16 = dg.tile([128, 128], f16, tag=f"cT16{k}", name=f"cT16{k}")
                nc.sync.dma_start_transpose(
                    c16[:], ctx_all[:, 128 * k:128 * k + 128])
                ctxT.append(c16)

            # ---- dec_in ----
            dps = p2.tile([128, G4], f32, tag="dz", bufs=1, name="dps")[:, 0:H]
            for k in range(3):
                nc.tensor.matmul(dps[:], ctxT[k][:], dinw[k][:],
                                 start=(k == 0), stop=False)
            nc.tensor.matmul(dps[:], tail[:], dinwt[:], start=False, stop=True)
            din_a = dg.tile([128, H], f16, tag="din_a")
            nc.scalar.activation(din_a[:], dps[:], AF.Relu)
            daT = []
            for k in range(3):
                pt = p2.tile([128, 128], f16, tag="sc2", bufs=2, name="pt")
                nc.tensor.transpose(pt[:], din_a[:, bass.ts(k, 128)], ident16[:])
                dt_ = dg.tile([128, 128], f16, tag=f"daT{k}", name=f"daT{k}")
                nc.scalar.copy(dt_[:], pt[:])
                daT.append(dt_)
            # ---- decoder LSTM ----
            zp = p2.tile([128, G4], f32, tag="dz", bufs=1, name="zp")
            for n in range(3):
                sl = slice(512 * n, 512 * (n + 1))
                nc.tensor.matmul(zp[:, sl], ones1[:], dbias[:, sl],
                                 start=True, stop=False)
                for k in range(3):
                    nc.tensor.matmul(zp[:, sl], daT[k][:], dwih[k][:, sl],
                                     start=False, stop=False)
                for k in range(3):
                    nc.tensor.matmul(zp[:, sl], rdT[k][:], dwhh[k][:, sl],
                                     start=False, stop=(k == 2))
            hd = gates(zp, cd, "d")
            for k in range(3):
                pt = p2.tile([128, 128], f16, tag="sc2", bufs=2, name="pt")
                nc.tensor.transpose(pt[:], hd[:, bass.ts(k, 128)], ident16[:])
                nc.scalar.copy(wrT[k][:], pt[:])
            # ---- output head ----
            yps = p2.tile([128, 512], f32, tag="sc2", bufs=2, name="yps")
            for k in range(3):
                nc.tensor.matmul(yps[:, 0:1], wrT[k][:], outw[k][:],
                                 start=(k == 0), stop=(k == 2))
            ybs = dg.tile([128, 1], f32, tag="ybs")
            nc.scalar.activation(ybs[:], yps[:, 0:1], AF.Identity,
                                 bias=obrep[:, 0:1])
            nc.sync.dma_start(d_y[:, bass.ds(t, 1)], ybs[:])
            ytp = p2.tile([128, 512], f32, tag="sc2", bufs=2, name="ytp")
            nc.tensor.transpose(ytp[0:1, 0:128], ybs[:], ident[:])
            nc.scalar.copy(tail[0:1, :], ytp[0:1, 0:128])

        for t in range(HZ):
            dec_step(t)

    nc.compile()
    return nc


def _perm_gates_rows(w):
    """Reorder LSTM gate rows from [i,f,g,o] to [i,f,o,g]. w: [4H, ...] or [4H]."""
    return np.concatenate([w[0:H], w[H:2 * H], w[3 * H:4 * H], w[2 * H:3 * H]], 0)


def _prep(inputs):
    """Host-side packing of all weights/inputs into device layouts."""
    g = {k: np.asarray(v, np.float32) for k, v in inputs.items()}
    h16 = lambda a: np.ascontiguousarray(a, dtype=np.float16)
    P = _perm_gates_rows
    pr = {}
    pr["wih0"] = h16(np.concatenate(
        [P(g["enc_Wih0"]).T, P(g["enc_bih0"] + g["enc_bhh0"])[None, :]], 0))
    pr["whh0"] = h16(P(g["enc_Whh0"]).T.reshape(3, 128, G4))
    pr["wih1"] = h16(P(g["enc_Wih1"]).T.reshape(3, 128, G4))
    pr["whh1"] = h16(P(g["enc_Whh1"]).T.reshape(3, 128, G4))
    pr["bias1"] = h16(P(g["enc_bih1"] + g["enc_bhh1"])[None, :])
    wa = g["Wa"]  # [dec_h, enc_h]
    pr["wa"] = h16(wa.reshape(3, 128, 3, 128).transpose(0, 2, 1, 3))
    W = g["dec_in_W"]; bvec = g["dec_in_b"]
    Wp = np.concatenate([W[:, 5:389], W[:, 0:1], W[:, 1:5], bvec[:, None]], 1)
    WpT = Wp.T  # [390, 384]
    pr["dinw"] = h16(WpT[:384].reshape(3, 128, H))
    pr["dinwt"] = h16(WpT[384:390])
    pr["dwih"] = h16(P(g["dec_Wih"]).T.reshape(3, 128, G4))
    pr["dwhh"] = h16(P(g["dec_Whh"]).T.reshape(3, 128, G4))
    pr["dbias"] = h16(P(g["dec_bih"] + g["dec_bhh"])[None, :])
    pr["outw"] = h16(g["out_W"].T.reshape(3, 128, 1))
    pr["outb"] = np.ascontiguousarray(g["out_b"].reshape(1, 1), np.float32)
    return g, pr


def _core_inputs(g, pr, c):
    sl = slice(c * BL, (c + 1) * BL)
    x = g["x"][sl]                     # [128, L, 8]
    xe = np.concatenate([x, np.ones((BL, L, 1), np.float32)], 2)
    m = dict(pr)
    m["xT"] = np.ascontiguousarray(xe.transpose(1, 2, 0), np.float16)
    m["featT"] = np.ascontiguousarray(
        g["future_feats"][sl].transpose(1, 2, 0), np.float16)
    return m


def kernel(**inputs):
    if "nc" not in _cache:
        _cache["nc"] = _build()
    nc = _cache["nc"]
    g, pr = _prep(inputs)
    in_maps = [_core_inputs(g, pr, c) for c in range(NC)]
    res = bass_utils.run_bass_kernel_spmd(nc, in_maps, core_ids=list(range(NC)))
    out = np.concatenate([res.results[c]["y"] for c in range(NC)], 0)
    return np.ascontiguousarray(out[:, :, None], np.float32)
